# revision 20
# baseline (speedup 1.0000x reference)
"""Trainium2 Bass kernel for nn_AttentionHead_80436147520097.

Single attention head, B=4 T=4096 D=1024 H=64:
    k,q,v = x@W+b;  S[t,s] = k_t . q_s / 8 (causal s<=t);  out = softmax_s(S) @ v

Sharding: 8 cores = 4 batches x 2 parity groups. Within a batch, the two
cores split the softmax (s) dimension by 128-row block parity: core p owns
s-blocks with (block_idx % 2 == p). Every t-chunk's causal extent is a
multiple of 4 blocks, so both parities get exactly half of every chunk's
work -> perfectly balanced AND structurally identical programs (true SPMD,
one NEFF). Odd-parity divergence is pushed into host-prepared data:
  - x rows are 128-block pair-swapped for p==1, so "even device blocks"
    are always the core's own s-blocks (device t order is permuted within
    512-aligned chunks; host un-permutes the output columns).
  - diagonal causal mask tiles are host-computed per parity.
Each core emits partial unnormalized out^T [65, T] (row 64 = softmax
denominator) over its s-half; host adds the two halves, divides, transposes.

On-chip dataflow (all matmul operands bf16, fp32 PSUM accumulation):
  host sends x already transposed+bf16 (xT [D,T]) -> contiguous DMA loads ->
  kT projection col-packed 2-up on the PE (even t-chunk -> psum parts 0:64,
  odd -> 64:128, giving the kT2 layout) + packed [Wq|Wv] projection (qvT
  over own s-cols) -> S^T[s,t] = qT.T @ kT per 128x512 block -> ACT
  exp(scale=1/8) -> bf16 -> (diagonal: mask multiply) -> PV:
  out^T[65,t] += [V|1].T @ expS^T, accumulated in PSUM over s.
"""

import sys

import numpy as np

try:
    import ml_dtypes
except ImportError:  # pragma: no cover
    sys.path.insert(0, "/opt/trn_rl_repo")
    import ml_dtypes

B, T, D, H = 4, 4096, 1024, 64
NCORES = 8
NCHUNK = 8          # t-chunks of 512 per batch
NTASK = 16          # s-tasks (128 cols) per core = T/128/2
BF16 = ml_dtypes.bfloat16

_cache = {}


def _build_program():
    import concourse.bacc as bacc
    import concourse.mybir as mybir
    import concourse.tile as tile

    f32 = mybir.dt.float32
    bf16 = mybir.dt.bfloat16

    nc = bacc.Bacc("TRN2", target_bir_lowering=False, debug=False,
                   num_devices=NCORES)

    xT_d = nc.dram_tensor("xT", [D, T], bf16, kind="ExternalInput").ap()
    wqq_d = nc.dram_tensor("wqq", [D, 128], bf16, kind="ExternalInput").ap()
    wk_d = nc.dram_tensor("wk", [D, H], bf16, kind="ExternalInput").ap()
    wv_d = nc.dram_tensor("wv", [D, H], bf16, kind="ExternalInput").ap()
    bqq_d = nc.dram_tensor("bqq", [128, 1], f32, kind="ExternalInput").ap()
    bvv_d = nc.dram_tensor("bvv", [128, 1], f32, kind="ExternalInput").ap()
    bkk_d = nc.dram_tensor("bkk", [128, 1], f32, kind="ExternalInput").ap()
    mask_d = nc.dram_tensor("mask", [2, 128, 512], bf16,
                            kind="ExternalInput").ap()
    out_d = nc.dram_tensor("out", [65, T], f32, kind="ExternalOutput").ap()

    with tile.TileContext(nc) as tc:
        with (
            tc.tile_pool(name="const", bufs=1) as const,
            tc.tile_pool(name="xT", bufs=1) as xT_p,
            tc.tile_pool(name="sb", bufs=1) as sb,
            tc.tile_pool(name="exp", bufs=6) as exp_p,
            tc.tile_pool(name="proj_ps", bufs=2, space="PSUM") as proj_ps,
            tc.tile_pool(name="spair_ps", bufs=2, space="PSUM") as spair_ps,
            tc.tile_pool(name="out_ps", bufs=2, space="PSUM") as out_ps,
        ):
            # ---- constants (wk first; xT u=0 loads are issued just after
            # it, before the remaining consts, so the PE can start ASAP) ----
            wk = const.tile([128, 8 * H], bf16)
            nc.sync.dma_start(
                wk[:].rearrange("p (c m) -> p c m", c=8),
                wk_d.rearrange("(c p) m -> p c m", p=128))
            xT = xT_p.tile([128, 8 * T], bf16)  # col c*T+s = x[s, c*128+p]
            xT3 = xT[:].rearrange("p (c s) -> p c s", c=8)
            xTd3 = xT_d.rearrange("(c p) s -> p c s", p=128)
            for c in range(8):
                nc.sync.dma_start(xT3[:, c, 0:1024], xTd3[:, c, 0:1024])
            wqq = const.tile([128, 8 * 128], bf16)
            nc.sync.dma_start(
                wqq[:].rearrange("p (c m) -> p c m", c=8),
                wqq_d.rearrange("(c p) m -> p c m", p=128))
            wv = const.tile([128, 8 * H], bf16)
            nc.sync.dma_start(
                wv[:].rearrange("p (c m) -> p c m", c=8),
                wv_d.rearrange("(c p) m -> p c m", p=128))
            bqq = const.tile([128, 1], f32)
            nc.sync.dma_start(bqq[:], bqq_d)
            bvv = const.tile([128, 1], f32)
            nc.sync.dma_start(bvv[:], bvv_d)
            bkk = const.tile([128, 1], f32)
            nc.sync.dma_start(bkk[:], bkk_d)
            masks = const.tile([128, 2 * 512], bf16)
            nc.sync.dma_start(
                masks[:].rearrange("p (m t) -> p m t", m=2),
                mask_d.rearrange("m p t -> p m t"))
            for u in range(1, 4):
                for c in range(8):
                    nc.sync.dma_start(xT3[:, c, 1024 * u:1024 * (u + 1)],
                                      xTd3[:, c, 1024 * u:1024 * (u + 1)])

            # ---- projections, interleaved per 1024-col s-range ----
            # kT2 layout: [0:64, u*512+t'] = k(chunk 2u), [64:128, ...] = 2u+1
            kT2 = sb.tile([128, 4 * 512], bf16)
            qqT = sb.tile([128, NTASK * 128], bf16)
            # vvT: [0:64, u*256+i] = v tasks 4u..4u+1 ; [64:128,...] = 4u+2..3
            vvT = sb.tile([128, 4 * 256], bf16)
            v_nat = sb.tile([128, NTASK * 80], bf16)
            ones_col = v_nat[:].rearrange("p (n w) -> p n w", w=80)[:, :, 64:65]
            nc.vector.memset(ones_col, 1.0)
            # prefetch ACT exp table set off the critical path
            scratch = const.tile([1, 8], f32)
            nc.vector.memset(scratch[:], 0.0)
            nc.scalar.activation(scratch[:], scratch[:],
                                 mybir.ActivationFunctionType.Exp)

            def proj_u(u):
                s0 = 1024 * u
                # kT for t-chunks 2u (psum rows 0:64) and 2u+1 (rows 64:128),
                # col-packed: two concurrent 128x64 PE tiles
                ps = proj_ps.tile([128, 512], f32, tag="proj")
                for c in range(8):
                    for h in range(2):
                        nc.tensor.matmul(
                            ps[64 * h: 64 * h + 64, :],
                            wk[:, c * H:(c + 1) * H],
                            xT3[:, c, s0 + 512 * h: s0 + 512 * (h + 1)],
                            start=(c == 0), stop=(c == 7),
                            skip_group_check=True,
                        )
                nc.vector.tensor_scalar_add(
                    kT2[:, u * 512:(u + 1) * 512], ps[:], bkk[:, 0:1])
                # qqT (q duplicated to both bands) for tasks [4u, 4u+4)
                ps2 = proj_ps.tile([128, 512], f32, tag="proj")
                for c in range(8):
                    rhs = xT3[:, c, s0:s0 + 1024]
                    rhs = rhs.rearrange("p (n two w) -> p n two w", two=2, w=128)
                    nc.tensor.matmul(
                        ps2[:], wqq[:, c * 128:(c + 1) * 128], rhs[:, :, 0:1, :],
                        start=(c == 0), stop=(c == 7),
                        skip_group_check=True,
                    )
                nc.vector.tensor_scalar_add(
                    qqT[:, u * 512:(u + 1) * 512], ps2[:], bqq[:, 0:1])
                # vvT per u, col-packed: tasks 4u..4u+1 -> band 0,
                # tasks 4u+2..4u+3 -> band 64 (two concurrent 128x64 tiles)
                ps3 = proj_ps.tile([128, 512], f32, tag="proj")
                for c in range(8):
                    rhs = xT3[:, c, s0:s0 + 1024]
                    rhs = rhs.rearrange("p (n two w) -> p n two w", two=2, w=128)
                    for h in range(2):
                        nc.tensor.matmul(
                            ps3[64 * h: 64 * h + 64, 0:256],
                            wv[:, c * H:(c + 1) * H],
                            rhs[:, 2 * h:2 * h + 2, 0:1, :],
                            start=(c == 0), stop=(c == 7),
                            skip_group_check=True,
                        )
                nc.vector.tensor_scalar_add(
                    vvT[:, u * 256:(u + 1) * 256], ps3[:, 0:256], bvv[:, 0:1])
                # v natural for tasks 4u..4u+3 (SBUF->SBUF DMA transpose)
                for ts in range(4 * u, 4 * u + 4):
                    band = 64 * ((ts % 4) // 2)
                    col = u * 256 + (ts % 2) * 128
                    nc.sync.dma_start(
                        out=v_nat[:, ts * 80: ts * 80 + 64],
                        in_=vvT[band:band + 64, col:col + 128],
                        transpose=True,
                    )

            # ---- attention, interleaved with projection stages ----
            outbuf = sb.tile([65, T], f32)

            def attn_chunk(j):
                po = out_ps.tile([65, 512], f32, tag="po")
                krow = 64 * (j % 2)
                kcol = (j // 2) * 512
                for tp in range(j + 1):
                    ps = spair_ps.tile([128, 1024], f32, tag="spair")
                    e = exp_p.tile([128, 1024], bf16, tag="exp")
                    for h in range(2):
                        ts = 2 * tp + h
                        nc.tensor.matmul(
                            ps[:, h * 512:(h + 1) * 512],
                            qqT[krow:krow + H, ts * 128:(ts + 1) * 128],
                            kT2[krow:krow + H, kcol:kcol + 512],
                            start=True, stop=True, skip_group_check=True,
                        )
                    nc.scalar.activation(
                        e[:], ps[:], mybir.ActivationFunctionType.Exp,
                        scale=0.125)
                    if tp == j:  # diagonal pair: causal mask
                        nc.vector.tensor_mul(
                            e[:, 0:512], e[:, 0:512], masks[:, 0:512])
                        nc.vector.tensor_mul(
                            e[:, 512:1024], e[:, 512:1024], masks[:, 512:1024])
                    for h in range(2):
                        ts = 2 * tp + h
                        nc.tensor.matmul(
                            po[:], v_nat[:, ts * 80: ts * 80 + 65],
                            e[:, h * 512:(h + 1) * 512],
                            start=(tp == 0 and h == 0),
                            stop=(tp == j and h == 1),
                            skip_group_check=True,
                        )
                nc.vector.tensor_copy(outbuf[:, j * 512:(j + 1) * 512], po[:])

            for u in range(4):
                proj_u(u)
                attn_chunk(2 * u)
                attn_chunk(2 * u + 1)
            for j in range(NCHUNK):
                nc.sync.dma_start(out_d[:, j * 512:(j + 1) * 512],
                                  outbuf[:, j * 512:(j + 1) * 512])
    nc.compile()
    return nc


def _host_masks():
    """mask[parity][m][s, t'] over device-t coords within a 512 chunk."""
    out = np.zeros((2, 2, 128, 512), dtype=BF16)
    s = np.arange(128)[:, None]
    tp = np.arange(512)[None, :]
    for p in range(2):
        if p == 0:
            t_orig = tp
        else:  # device blocks pair-swapped
            t_orig = ((tp // 128) ^ 1) * 128 + tp % 128
        for m in range(2):
            s_orig = (2 * m + p) * 128 + s
            out[p, m] = (s_orig <= t_orig).astype(BF16)
    return out


def kernel(x, Wk, bk, Wq, bq, Wv, bv):
    from concourse.bass_utils import run_bass_kernel_spmd

    if "nc" not in _cache:
        _cache["nc"] = _build_program()
    nc = _cache["nc"]

    x = np.asarray(x, np.float32)
    wqq = np.concatenate([np.asarray(Wq), np.asarray(Wq)], axis=1).astype(BF16)
    wk_np = np.asarray(Wk).astype(BF16)
    wv_np = np.asarray(Wv).astype(BF16)
    dup = lambda v: np.concatenate([np.asarray(v), np.asarray(v)]).astype(
        np.float32).reshape(128, 1)
    masks = _host_masks()

    in_maps = []
    for core in range(NCORES):
        b, p = core // 2, core % 2
        xb = x[b]
        if p == 1:  # swap 128-row blocks within 256-row pairs
            xb = xb.reshape(T // 256, 2, 128, D)[:, ::-1].reshape(T, D)
        in_maps.append({
            "xT": np.ascontiguousarray(xb.T.astype(BF16)),
            "wqq": wqq,
            "wk": wk_np,
            "wv": wv_np,
            "bqq": dup(bq),
            "bvv": dup(bv),
            "bkk": dup(bk),
            "mask": np.ascontiguousarray(masks[p]),
        })

    res = run_bass_kernel_spmd(nc, in_maps, core_ids=list(range(NCORES)))
    results = res.results
    _cache["last_run"] = res

    out = np.zeros((B, T, H), np.float32)
    for b in range(B):
        a0 = results[2 * b]["out"]        # [65, T] device-t natural
        a1 = results[2 * b + 1]["out"]    # [65, T] device-t pair-swapped
        a1 = a1.reshape(65, T // 256, 2, 128)[:, :, ::-1].reshape(65, T)
        tot = a0 + a1
        out[b] = (tot[0:64] / tot[64:65]).T
    return out
